# revision 9
# baseline (speedup 1.0000x reference)
"""Trainium2 Bass kernel for a 2-layer autoregressive transformer.

Sharding: pure data-parallel over batch. B=16 sequences are split across
8 NeuronCores (2 sequences / core). Each core runs the full forward pass
(embedding gather, 2 transformer layers, final LN, lm_head) on its shard
and writes its [2048, 10000] slice of the logits; the host concatenates
the 8 slices.

Device-side layout notes:
 - tokens live on SBUF partitions in blocks of 128; per core there are
   NB=16 blocks (2 seqs x 8 blocks).
 - attention uses transposed scores (scoresT[s, q] = k @ q^T per head) so
   the exp'd tile is directly the lhsT for the P @ V matmul - no
   transposes of P needed.  The softmax denominator comes for free from a
   ones-column appended to V.
 - lm_head streams w_head [128, 10000] from SBUF as the moving operand in
   512-wide slices.
"""

import sys

sys.path.insert(0, "/opt/trn_rl_repo")

import numpy as np

import concourse.bass as bass
import concourse.bacc as bacc
import concourse.mybir as mybir
from concourse.tile import TileContext
from concourse.bass_utils import run_bass_kernel_spmd
from concourse.masks import make_identity

F32 = mybir.dt.float32
I32 = mybir.dt.int32
AF = mybir.ActivationFunctionType
ALU = mybir.AluOpType

B, T, V, C, H, HS, FF, L = 16, 1024, 10000, 128, 4, 32, 512, 2
NCORES = 8
BL = B // NCORES            # sequences per core
NTOK = BL * T               # tokens per core
NB = NTOK // 128            # 16 token blocks per core
NSB = T // 128              # 8 blocks per sequence
EPS = 1e-5
SCALE = C ** -0.5
VSL = 512                   # vocab slice for lm_head
NVS = (V + VSL - 1) // VSL  # 20 slices (last is 272)

_CACHED_NC = None


def _emit(nc, tc, t_in, t_out):
    from contextlib import ExitStack
    stack = ExitStack()
    pp = stack.enter_context(tc.tile_pool(name="persist", bufs=1))
    wp = stack.enter_context(tc.tile_pool(name="work", bufs=3))
    ptp = stack.enter_context(tc.tile_pool(name="ptpool", bufs=36))
    stp = stack.enter_context(tc.tile_pool(name="stage", bufs=4))
    ps128 = stack.enter_context(tc.tile_pool(name="ps128", bufs=4, space="PSUM"))
    ps512 = stack.enter_context(tc.tile_pool(name="ps512", bufs=2, space="PSUM"))
    psat = stack.enter_context(tc.tile_pool(name="psat", bufs=2, space="PSUM"))

    # ---- constants / small inputs ----
    ident = pp.tile([128, 128], F32, name="ident")
    make_identity(nc, ident)
    cm_sb = pp.tile([128, 128], F32, name="cm_sb")
    nc.sync.dma_start(out=cm_sb, in_=t_in["cmask"][:, :])
    ones1 = pp.tile([1, 128], F32, name="ones1")
    nc.vector.memset(ones1, 1.0)
    eps_sb = pp.tile([128, 1], F32, name="eps_sb")
    nc.vector.memset(eps_sb, EPS)
    idx_sb = pp.tile([128, NB], I32, name="idx_sb")
    nc.sync.dma_start(out=idx_sb, in_=t_in["idx"][:, :])
    pos_sb = pp.tile([128, NSB, C], F32, name="pos_sb")
    nc.sync.dma_start(out=pos_sb, in_=t_in["pos"][:, :, :])

    def bcast_load(name):
        tile = pp.tile([128, C], F32, name=name + "_bc")
        src = t_in[name]
        ap = bass.AP(tensor=src.tensor if isinstance(src, bass.AP) else src,
                     offset=0, ap=[[0, 128], [1, C]])
        nc.sync.dma_start(out=tile, in_=ap)
        return tile

    # ---- per layer weights ----
    wq_sb, wk_sb, wv_sb, wpj_sb = [], [], [], []
    w1_sb, w2_sb, bf1_sb = [], [], []
    bp_sb, bf2_sb = [], []
    g1_bc, b1_bc, g2_bc, b2_bc = [], [], [], []
    for l in range(L):
        for lst, nm, shp in ((wq_sb, f"wq{l}", [C, C]), (wk_sb, f"wk{l}", [C, C]),
                             (wv_sb, f"wv{l}", [C, C]), (wpj_sb, f"wp{l}", [C, C]),
                             (w1_sb, f"w1{l}", [C, FF]), (w2_sb, f"w2{l}", [C, 4, C]),
                             (bf1_sb, f"bf1{l}", [C, 4]), (bp_sb, f"bp{l}", [1, C]),
                             (bf2_sb, f"bf2{l}", [1, C])):
            tile = pp.tile(shp, F32, name=nm + "_sb")
            nc.sync.dma_start(out=tile, in_=t_in[nm][...])
            lst.append(tile)
        g1_bc.append(bcast_load(f"g1{l}"))
        b1_bc.append(bcast_load(f"b1{l}"))
        g2_bc.append(bcast_load(f"g2{l}"))
        b2_bc.append(bcast_load(f"b2{l}"))
    gf_bc = bcast_load("gf")
    bf_bc = bcast_load("bf")

    # ---- persistent activations ----
    x_all = pp.tile([128, NB, C], F32, name="x_all")
    hT_all = pp.tile([128, NTOK], F32, name="hT_all")
    # q/k transposed, split into head-pair tiles so per-head partition
    # slices start at base partition 0 or 32 (matmul xbus constraint).
    qT_p = [pp.tile([64, NTOK], F32, name=f"qT_p{p}") for p in range(2)]
    kT_p = [pp.tile([64, NTOK], F32, name=f"kT_p{p}") for p in range(2)]
    v_ext = pp.tile([128, NB, H, HS + 1], F32, name="v_ext")
    ff1T = pp.tile([128, 4, NTOK], F32, name="ff1T")
    nc.vector.memset(v_ext[:, :, :, HS:HS + 1], 1.0)

    # ---- embedding gather ----
    for n in range(NB):
        xg = wp.tile([128, C], F32, name="xg")
        nc.gpsimd.indirect_dma_start(
            out=xg[:, :], out_offset=None, in_=t_in["tok_emb"][:, :],
            in_offset=bass.IndirectOffsetOnAxis(ap=idx_sb[:, n:n + 1], axis=0))
        nc.vector.tensor_add(out=x_all[:, n, :], in0=xg[:, :],
                             in1=pos_sb[:, n % NSB, :])

    # lm_head weights loaded after the embedding DMAs are queued
    wh_sb = pp.tile([128, V], F32, name="wh_sb")
    nc.sync.dma_start(out=wh_sb, in_=t_in["wh"][:, :])

    def layer_norm(x_ap, g_bc, b_bc, h_out):
        st = wp.tile([128, 6], F32, name="st")
        nc.vector.bn_stats(out=st, in_=x_ap)
        mv = wp.tile([128, 2], F32, name="mv")
        nc.vector.bn_aggr(out=mv, in_=st)
        sd = wp.tile([128, 1], F32, name="sd")
        nc.scalar.activation(out=sd, in_=mv[:, 1:2], func=AF.Sqrt,
                             bias=eps_sb[:, :], scale=1.0)
        rstd = wp.tile([128, 1], F32, name="rstd")
        nc.vector.reciprocal(out=rstd, in_=sd)
        nc.vector.tensor_scalar(out=h_out, in0=x_ap, scalar1=mv[:, 0:1],
                                scalar2=rstd, op0=ALU.subtract, op1=ALU.mult)
        nc.vector.tensor_mul(out=h_out, in0=h_out, in1=g_bc)
        nc.vector.tensor_add(out=h_out, in0=h_out, in1=b_bc)

    def transpose_to(dst_ap, src_ap):
        tp = ps128.tile([128, 128], F32, name="tp", tag="ps128")
        nc.tensor.transpose(out=tp, in_=src_ap, identity=ident)
        nc.scalar.copy(out=dst_ap, in_=tp)

    for l in range(L):
        # LN1 -> hT
        for n in range(NB):
            h = wp.tile([128, C], F32, name="h")
            layer_norm(x_all[:, n, :], g1_bc[l], b1_bc[l], h)
            transpose_to(hT_all[:, n * 128:(n + 1) * 128], h)
        # qT / kT (512-wide), v (per block)
        for g in range(NTOK // 512):
            sl5 = slice(g * 512, (g + 1) * 512)
            pq = ps512.tile([128, 512], F32, name="pq", tag="ps512")
            nc.tensor.matmul(pq, lhsT=wq_sb[l], rhs=hT_all[:, sl5],
                             start=True, stop=True)
            nc.scalar.mul(out=qT_p[0][:, sl5], in_=pq[0:64, :], mul=SCALE)
            nc.scalar.mul(out=qT_p[1][:, sl5], in_=pq[64:128, :], mul=SCALE)
            pk = ps512.tile([128, 512], F32, name="pk", tag="ps512")
            nc.tensor.matmul(pk, lhsT=wk_sb[l], rhs=hT_all[:, sl5],
                             start=True, stop=True)
            nc.scalar.copy(out=kT_p[0][:, sl5], in_=pk[0:64, :])
            nc.scalar.copy(out=kT_p[1][:, sl5], in_=pk[64:128, :])
        for n in range(NB):
            pv = ps128.tile([128, 128], F32, name="pv", tag="ps128")
            nc.tensor.matmul(pv, lhsT=hT_all[:, n * 128:(n + 1) * 128], rhs=wv_sb[l],
                             start=True, stop=True)
            nc.scalar.copy(out=v_ext[:, n, :, 0:HS],
                           in_=pv.rearrange("p (h e) -> p h e", e=HS))
        # attention
        for s in range(BL):
            for i in range(NSB):
                nq = s * NSB + i
                aps = psat.tile([128, H, HS + 1], F32, name="aps", tag="aps")
                n_mm = H * (i + 1)
                pts = {}
                for hh in range(H):
                    for j in range(i + 1):
                        nk = s * NSB + j
                        sc = ps128.tile([128, 128], F32, name="sc", tag="ps128")
                        pb = 32 * (hh % 2)
                        nc.tensor.matmul(
                            sc,
                            lhsT=kT_p[hh // 2][pb:pb + 32, nk * 128:(nk + 1) * 128],
                            rhs=qT_p[hh // 2][pb:pb + 32, nq * 128:(nq + 1) * 128],
                            start=True, stop=True)
                        if j == i:
                            nc.vector.tensor_add(out=sc, in0=sc, in1=cm_sb)
                        pt = ptp.tile([128, 128], F32, name="pt")
                        nc.scalar.activation(out=pt, in_=sc, func=AF.Exp)
                        pts[(hh, j)] = pt
                k_mm = 0
                for hh in range(H):
                    for j in range(i + 1):
                        nk = s * NSB + j
                        nc.tensor.matmul(
                            aps[:, hh, :], lhsT=pts[(hh, j)],
                            rhs=v_ext[:, nk, hh, :],
                            start=(k_mm == 0), stop=(k_mm == n_mm - 1))
                        k_mm += 1
                att = wp.tile([128, C], F32, name="att")
                zr = wp.tile([128, H], F32, name="zr")
                nc.vector.reciprocal(out=zr, in_=aps[:, :, HS])
                for hh in range(H):
                    nc.vector.tensor_scalar_mul(
                        out=att[:, HS * hh:HS * hh + HS],
                        in0=aps[:, hh, 0:HS], scalar1=zr[:, hh:hh + 1])
                attT = wp.tile([128, C], F32, name="attT")
                transpose_to(attT, att)
                pr = ps128.tile([128, 128], F32, name="pr", tag="ps128")
                nc.tensor.matmul(pr, lhsT=attT, rhs=wpj_sb[l], start=True, stop=False)
                nc.tensor.matmul(pr, lhsT=ones1, rhs=bp_sb[l], start=False, stop=True)
                nc.vector.tensor_add(out=x_all[:, nq, :], in0=x_all[:, nq, :], in1=pr)
        # LN2 -> hT
        for n in range(NB):
            h = wp.tile([128, C], F32, name="h")
            layer_norm(x_all[:, n, :], g2_bc[l], b2_bc[l], h)
            transpose_to(hT_all[:, n * 128:(n + 1) * 128], h)
        # ff1 (transposed) with fused bias+relu
        for g in range(NTOK // 512):
            for sl in range(4):
                pf = ps512.tile([128, 512], F32, name="pf", tag="ps512")
                nc.tensor.matmul(pf, lhsT=w1_sb[l][:, sl * 128:(sl + 1) * 128],
                                 rhs=hT_all[:, g * 512:(g + 1) * 512],
                                 start=True, stop=True)
                nc.scalar.activation(out=ff1T[:, sl, g * 512:(g + 1) * 512], in_=pf,
                                     func=AF.Relu, bias=bf1_sb[l][:, sl:sl + 1],
                                     scale=1.0)
        # ff2 + bias + residual
        for n in range(NB):
            p2 = ps128.tile([128, 128], F32, name="p2", tag="ps128")
            for sl in range(4):
                nc.tensor.matmul(p2, lhsT=ff1T[:, sl, n * 128:(n + 1) * 128],
                                 rhs=w2_sb[l][:, sl, :],
                                 start=(sl == 0), stop=False)
            nc.tensor.matmul(p2, lhsT=ones1, rhs=bf2_sb[l], start=False, stop=True)
            nc.vector.tensor_add(out=x_all[:, n, :], in0=x_all[:, n, :], in1=p2)

    # final LN + lm_head
    for n in range(NB):
        hf = wp.tile([128, C], F32, name="hf")
        layer_norm(x_all[:, n, :], gf_bc, bf_bc, hf)
        hfT = wp.tile([128, C], F32, name="hfT")
        transpose_to(hfT, hf)
        for vs in range(NVS):
            v0 = vs * VSL
            vsz = min(VSL, V - v0)
            ph = ps512.tile([128, VSL], F32, name="ph", tag="ps512")
            nc.tensor.matmul(ph[:, :vsz], lhsT=hfT, rhs=wh_sb[:, v0:v0 + vsz],
                             start=True, stop=True)
            so = stp.tile([128, VSL], F32, name="so")
            if vs % 2 == 0:
                nc.scalar.copy(out=so[:, :vsz], in_=ph[:, :vsz])
            else:
                nc.vector.tensor_copy(out=so[:, :vsz], in_=ph[:, :vsz])
            nc.sync.dma_start(out=t_out[n * 128:(n + 1) * 128, v0:v0 + vsz],
                              in_=so[:, :vsz])
    stack.close()


def build_module():
    global _CACHED_NC
    if _CACHED_NC is not None:
        return _CACHED_NC
    nc = bacc.Bacc("TRN2", target_bir_lowering=False, debug=False)
    t_in = {}
    t_in["idx"] = nc.declare_dram_parameter("idx", [128, NB], I32, isOutput=False)
    t_in["tok_emb"] = nc.declare_dram_parameter("tok_emb", [V, C], F32, isOutput=False)
    t_in["pos"] = nc.declare_dram_parameter("pos", [128, NSB, C], F32, isOutput=False)
    t_in["cmask"] = nc.declare_dram_parameter("cmask", [128, 128], F32, isOutput=False)
    for l in range(L):
        for nm, shp in ((f"wq{l}", [C, C]), (f"wk{l}", [C, C]), (f"wv{l}", [C, C]),
                        (f"wp{l}", [C, C]), (f"w1{l}", [C, FF]), (f"w2{l}", [C, 4, C]),
                        (f"bf1{l}", [C, 4]), (f"bp{l}", [1, C]), (f"bf2{l}", [1, C]),
                        (f"g1{l}", [C]), (f"b1{l}", [C]), (f"g2{l}", [C]),
                        (f"b2{l}", [C])):
            t_in[nm] = nc.declare_dram_parameter(nm, shp, F32, isOutput=False)
    t_in["gf"] = nc.declare_dram_parameter("gf", [C], F32, isOutput=False)
    t_in["bf"] = nc.declare_dram_parameter("bf", [C], F32, isOutput=False)
    t_in["wh"] = nc.declare_dram_parameter("wh", [C, V], F32, isOutput=False)
    t_out = nc.declare_dram_parameter("out", [NTOK, V], F32, isOutput=True)

    with TileContext(nc) as tc:
        _emit(nc, tc, t_in, t_out)
    nc.finalize()
    _CACHED_NC = nc
    return nc


def prep_inputs(inputs):
    """Host-side shard prep: returns (in_maps for 8 cores, b_head)."""
    f32 = lambda a: np.ascontiguousarray(np.asarray(a), dtype=np.float32)
    idx = np.asarray(inputs["idx"]).astype(np.int32)

    shared = {
        "tok_emb": f32(inputs["tok_emb"]),
        "pos": f32(inputs["pos_emb"])[:T].reshape(NSB, 128, C).transpose(1, 0, 2).copy(),
        "cmask": np.where(np.arange(128)[None, :] >= np.arange(128)[:, None],
                          np.float32(0.0), np.float32(-1e30)).astype(np.float32),
        "gf": f32(inputs["lnf_g"]),
        "bf": f32(inputs["lnf_b"]),
        "wh": f32(inputs["w_head"]),
    }
    wq, wk, wv = inputs["wq"], inputs["wk"], inputs["wv"]
    for l in range(L):
        shared[f"wq{l}"] = f32(np.transpose(np.asarray(wq)[l], (1, 0, 2)).reshape(C, C))
        shared[f"wk{l}"] = f32(np.transpose(np.asarray(wk)[l], (1, 0, 2)).reshape(C, C))
        shared[f"wv{l}"] = f32(np.transpose(np.asarray(wv)[l], (1, 0, 2)).reshape(C, C))
        shared[f"wp{l}"] = f32(inputs["w_proj"][l])
        shared[f"w1{l}"] = f32(inputs["w_ff1"][l])
        shared[f"w2{l}"] = f32(np.asarray(inputs["w_ff2"][l]).reshape(4, 128, C)
                               .transpose(1, 0, 2))
        shared[f"bf1{l}"] = f32(np.asarray(inputs["b_ff1"][l]).reshape(4, 128).T)
        shared[f"bp{l}"] = f32(inputs["b_proj"][l]).reshape(1, C)
        shared[f"bf2{l}"] = f32(inputs["b_ff2"][l]).reshape(1, C)
        shared[f"g1{l}"] = f32(inputs["ln1_g"][l])
        shared[f"b1{l}"] = f32(inputs["ln1_b"][l])
        shared[f"g2{l}"] = f32(inputs["ln2_g"][l])
        shared[f"b2{l}"] = f32(inputs["ln2_b"][l])

    in_maps = []
    for c in range(NCORES):
        shard = idx[c * BL:(c + 1) * BL].reshape(NTOK)
        m = dict(shared)
        m["idx"] = np.ascontiguousarray(shard.reshape(NB, 128).T)
        in_maps.append(m)
    return in_maps, np.asarray(inputs["b_head"], dtype=np.float32)


def kernel(**inputs):
    nc = build_module()
    in_maps, b_head = prep_inputs(inputs)
    res = run_bass_kernel_spmd(nc, in_maps, core_ids=list(range(NCORES)))
    out = np.concatenate([r["out"] for r in res.results], axis=0)
    out = out.reshape(B, T, V)
    if np.any(b_head):
        out = out + b_head
    return out


# revision 50
# speedup vs baseline: 43.9477x; 43.9477x over previous
"""Trainium2 Bass kernel for a 2-layer autoregressive transformer.

Sharding: pure data-parallel over batch. B=16 sequences split across 8
NeuronCores (2 sequences / core). Each core runs the full forward pass
(embedding gather, 2 transformer layers, final LN, lm_head) on its shard
and writes its [2048, 10000] slice of the logits; the host concatenates
the 8 slices.  No collectives needed.

Device-side design:
 - tokens live on SBUF partitions in blocks of 128; 16 blocks / core.
 - attention computes transposed scores (scoresT[s, q] = k @ q^T per
   head) in wide spans, so the exp'd tile is directly the lhsT of the
   P @ V matmul - P never needs a transpose.  The softmax denominator
   comes free from a ones-column appended to V.  Causality is a 0/1
   multiply of the diagonal block after exp (on the idle gpsimd).
 - layer-norm gain/bias fold into the PSUM->SBUF copy after the h
   transpose as per-partition scalars (channels sit on partitions).
 - the pipeline runs per sequence and per causal half-sequence, so the
   first lm_head logits are ready ~1/4 into the kernel; finished lm_head
   blocks are deferred and "pumped" between later compute stages to keep
   the 82MB store stream (the roofline bottleneck) continuously fed.
   Stores alternate between HWDGE (SP) and SWDGE (gpsimd) queues.
 - fp32r (TF32-like, 4x faster at N>=256) is used for q/k/v/ff1
   projections, scores and the lm_head (LM_MODE / SCORES_F32R switches).
"""

import sys

sys.path.insert(0, "/opt/trn_rl_repo")

import numpy as np

import concourse.bass as bass
import concourse.bacc as bacc
import concourse.mybir as mybir
from concourse.tile import TileContext
from concourse.bass_utils import run_bass_kernel_spmd
from concourse.masks import make_identity

F32 = mybir.dt.float32
F32R = mybir.dt.float32r
I32 = mybir.dt.int32
AF = mybir.ActivationFunctionType
ALU = mybir.AluOpType

B, T, V, C, H, HS, FF, L = 16, 1024, 10000, 128, 4, 32, 512, 2
NCORES = 8
BL = B // NCORES            # sequences per core
NTOK = BL * T               # tokens per core
NB = NTOK // 128            # 16 token blocks per core
NSB = T // 128              # 8 blocks per sequence
EPS = 1e-5
SCALE = C ** -0.5
VSL = 512                   # lm_head matmul slice
DSL = 1024                  # logits DMA slice (2 matmul slices)

import os
SCORES_F32R = os.environ.get("K_SCORES_F32R", "1") == "1"
LM_MODE = os.environ.get("K_LM_MODE", "f32r")    # "f32" | "f32r"

_CACHED_NC = None


def _emit(nc, tc, t_in, t_out):
    from contextlib import ExitStack
    PDT = F32R if SCORES_F32R else F32      # projection-path dtype
    LDT = F32R if LM_MODE == "f32r" else F32
    stack = ExitStack()
    pp = stack.enter_context(tc.tile_pool(name="persist", bufs=1))
    wp = stack.enter_context(tc.tile_pool(name="work", bufs=3))
    ep = stack.enter_context(tc.tile_pool(name="expool", bufs=2))
    ap_ = stack.enter_context(tc.tile_pool(name="attpool", bufs=2))
    fp = stack.enter_context(tc.tile_pool(name="ffpool", bufs=2))
    hp = stack.enter_context(tc.tile_pool(name="hfpool", bufs=12))
    stp = stack.enter_context(tc.tile_pool(name="stage", bufs=3))
    ps128 = stack.enter_context(tc.tile_pool(name="ps128", bufs=2, space="PSUM"))
    ps512 = stack.enter_context(tc.tile_pool(name="ps512", bufs=2, space="PSUM"))
    pssc = stack.enter_context(tc.tile_pool(name="pssc", bufs=2, space="PSUM"))
    psat = stack.enter_context(tc.tile_pool(name="psat", bufs=2, space="PSUM"))

    # ---- constants / small inputs ----
    ident = pp.tile([128, 128], F32, name="ident")
    make_identity(nc, ident)
    cm_sb = pp.tile([128, 128], F32, name="cm_sb")
    nc.sync.dma_start(out=cm_sb, in_=t_in["cmask"][:, :])
    ones1 = pp.tile([1, 128], F32, name="ones1")
    nc.vector.memset(ones1, 1.0)
    eps_sb = pp.tile([128, 1], F32, name="eps_sb")
    nc.vector.memset(eps_sb, EPS)
    idx_sb = pp.tile([128, NB], I32, name="idx_sb")
    nc.sync.dma_start(out=idx_sb, in_=t_in["idx"][:, :])
    t_in_pos_sb = None
    if os.environ.get("K_EMBED", "new") in ("old", "batchgather"):
        t_in_pos_sb = pp.tile([128, NSB, C], F32, name="pos_sb")
        nc.sync.dma_start(out=t_in_pos_sb, in_=t_in["pos"][:, 0:NSB, :])

    def col_load(name):
        """[C] DRAM vector -> [128, 1] sbuf column (per-partition scalar)."""
        tile = pp.tile([128, 1], F32, name=name + "_col")
        src = t_in[name]
        apx = bass.AP(tensor=src.tensor if isinstance(src, bass.AP) else src,
                      offset=0, ap=[[1, C], [0, 1]])
        nc.sync.dma_start(out=tile, in_=apx)
        return tile

    # ---- per layer weights ----
    wq_sb, wk_sb, wv_sb, wpj_sb = [], [], [], []
    w1_sb, w2_sb, bf1_sb = [], [], []
    bp_sb, bf2_sb = [], []
    g1c, b1c, g2c, b2c = [], [], [], []
    for l in range(L):
        for lst, nm, shp, dt in (
                (wq_sb, f"wq{l}", [C, C], PDT), (wk_sb, f"wk{l}", [C, C], PDT),
                (wv_sb, f"wv{l}", [C, C], PDT), (wpj_sb, f"wp{l}", [C, C], F32),
                (w1_sb, f"w1{l}", [C, FF], PDT), (w2_sb, f"w2{l}", [C, 4, C], F32),
                (bf1_sb, f"bf1{l}", [C, 4], F32), (bp_sb, f"bp{l}", [1, C], F32),
                (bf2_sb, f"bf2{l}", [1, C], F32)):
            tile = pp.tile(shp, dt, name=nm + "_sb")
            nc.sync.dma_start(out=tile, in_=t_in[nm][...])
            lst.append(tile)
        g1c.append(col_load(f"g1{l}"))
        b1c.append(col_load(f"b1{l}"))
        g2c.append(col_load(f"g2{l}"))
        b2c.append(col_load(f"b2{l}"))
    gfc = col_load("gf")
    bfc = col_load("bf")

    # ---- persistent activations ----
    x_all = pp.tile([128, NB, C], F32, name="x_all")
    hT_all = pp.tile([128, NTOK], PDT, name="hT_all")
    # q/k transposed, head-pair tiles: per-head slices start at partition
    # 0 or 32 (matmul base-partition constraint).  The first causal half
    # of each sequence gets two layer-parity column regions so layer l+1
    # half 0 can overlap layer l half 1 (wavefront pipeline):
    # per seq: [h0 parity0 | h0 parity1 | h1] = 1536 cols.
    QKW = BL * 1536
    qT_p = [pp.tile([64, QKW], PDT, name=f"qT_p{p}") for p in range(2)]
    kT_p = [pp.tile([64, QKW], PDT, name=f"kT_p{p}") for p in range(2)]
    # v likewise: slots 0..15 = (s, blk); 16..23 = parity-1 copies of the
    # half-0 blocks.
    v_ext = pp.tile([128, NB + 8, H, HS + 1], F32, name="v_ext")
    nc.vector.memset(v_ext[:, :, :, HS:HS + 1], 1.0)

    def q_col(l, s, blk):
        if blk < 4:
            return s * 1536 + (l % 2) * 512 + blk * 128
        return s * 1536 + 1024 + (blk - 4) * 128

    def v_slot(l, s, blk):
        if blk < 4 and (l % 2) == 1:
            return NB + s * 4 + blk
        return s * NSB + blk

    # ---- embedding: token gather + positional add ----
    emb_mode = os.environ.get("K_EMBED", "new")
    if emb_mode == "batchgather":
        # batched gather, DVE pos add
        for c in range(2):
            nc.gpsimd.indirect_dma_start(
                out=x_all[:, c * 8:(c + 1) * 8, :], out_offset=None,
                in_=t_in["tok_emb"][:, :],
                in_offset=bass.IndirectOffsetOnAxis(
                    ap=idx_sb[:, c * 8:(c + 1) * 8], axis=0))
        for n in range(NB):
            nc.vector.tensor_add(out=x_all[:, n, :], in0=x_all[:, n, :],
                                 in1=t_in_pos_sb[:, n % NSB, :])
    elif emb_mode == "accum":
        # per-block gather, accum-DMA pos add
        for n in range(NB):
            nc.gpsimd.indirect_dma_start(
                out=x_all[:, n, :], out_offset=None, in_=t_in["tok_emb"][:, :],
                in_offset=bass.IndirectOffsetOnAxis(ap=idx_sb[:, n:n + 1],
                                                    axis=0))
            nc.gpsimd.dma_start(out=x_all[:, n, :],
                                in_=t_in["pos"][:, n, :], accum_op=ALU.add)
    elif emb_mode == "new":
        # per-block gathers (multi-index gather mis-executes on HW) with one
        # accumulate-DMA per 8-block half for the positional add
        for c in range(2):
            for n in range(c * 8, (c + 1) * 8):
                nc.gpsimd.indirect_dma_start(
                    out=x_all[:, n, :], out_offset=None,
                    in_=t_in["tok_emb"][:, :],
                    in_offset=bass.IndirectOffsetOnAxis(ap=idx_sb[:, n:n + 1],
                                                        axis=0))
            nc.gpsimd.dma_start(out=x_all[:, c * 8:(c + 1) * 8, :],
                                in_=t_in["pos"][:, c * 8:(c + 1) * 8, :],
                                accum_op=ALU.add)
    else:
        for n in range(NB):
            xg = wp.tile([128, C], F32, name="xg")
            nc.gpsimd.indirect_dma_start(
                out=xg[:, :], out_offset=None, in_=t_in["tok_emb"][:, :],
                in_offset=bass.IndirectOffsetOnAxis(ap=idx_sb[:, n:n + 1],
                                                    axis=0))
            nc.vector.tensor_add(out=x_all[:, n, :], in0=xg[:, :],
                                 in1=t_in_pos_sb[:, n % NSB, :])

    # lm_head weights: queued after embedding so they overlap layer compute
    wh_sb = pp.tile([128, V], LDT, name="wh_sb")
    nc.sync.dma_start(out=wh_sb, in_=t_in["wh"][:, :])

    def ln_norm_blocks(blocks, tag):
        """Normalized (x-m)*rstd for given blocks of x_all -> list of [128,C]
        sbuf tiles (gain/bias fold into the transposed copy later)."""
        nbl = len(blocks)
        mv_all = wp.tile([128, len(blocks), 2], F32, name="mv_" + tag, tag="mv")
        for k, n in enumerate(blocks):
            st = wp.tile([128, 6], F32, name="st", tag="st")
            nc.vector.bn_stats(out=st, in_=x_all[:, n, :])
            nc.vector.bn_aggr(out=mv_all[:, k, :], in_=st)
        sd = wp.tile([128, nbl], F32, name="sd_" + tag, tag="sd")
        nc.scalar.activation(out=sd, in_=mv_all[:, :, 1], func=AF.Sqrt,
                             bias=eps_sb[:, :], scale=1.0)
        rstd = wp.tile([128, nbl], F32, name="rs_" + tag, tag="rs")
        nc.vector.reciprocal(out=rstd, in_=sd)
        outs = []
        for k, n in enumerate(blocks):
            h = wp.tile([128, C], F32, name="h", tag="h")
            nc.vector.tensor_scalar(out=h, in0=x_all[:, n, :],
                                    scalar1=mv_all[:, k, 0:1],
                                    scalar2=rstd[:, k:k + 1],
                                    op0=ALU.subtract, op1=ALU.mult)
            outs.append(h)
        return outs

    def transpose_gb(dst_ap, src_ap, g_col, b_col):
        """dst[c, t] = src[t, c].T * g[c] + b[c]: PE transpose + fused
        per-partition scale/bias on the PSUM->SBUF copy (DVE)."""
        tp = ps128.tile([128, 128], F32, name="tp", tag="ps128")
        nc.tensor.transpose(out=tp, in_=src_ap, identity=ident)
        nc.vector.tensor_scalar(out=dst_ap, in0=tp, scalar1=g_col,
                                scalar2=b_col, op0=ALU.mult, op1=ALU.add)

    def transpose_to(dst_ap, src_ap, engine_pick=0):
        tp = ps128.tile([128, 128], F32, name="tp", tag="ps128")
        nc.tensor.transpose(out=tp, in_=src_ap, identity=ident)
        if engine_pick == 0:
            nc.scalar.copy(out=dst_ap, in_=tp)
        else:
            nc.vector.tensor_copy(out=dst_ap, in_=tp)

    # ---- deferred lm_head work queue (one DMA slice per task) ----
    lm_pending = []

    def lm_slice(n, hfT, d0):
        dsz = min(DSL, V - d0)
        so = stp.tile([128, DSL], F32, name="so")
        for c0 in range(d0, d0 + dsz, VSL):
            vsz = min(VSL, V - c0)
            ph = ps512.tile([128, VSL], F32, name="ph", tag="ps512")
            nc.tensor.matmul(ph[:, :vsz], lhsT=hfT,
                             rhs=wh_sb[:, c0:c0 + vsz],
                             start=True, stop=True)
            if (c0 // VSL) % 2 == 0:
                nc.scalar.copy(out=so[:, c0 - d0:c0 - d0 + vsz],
                               in_=ph[:, :vsz])
            else:
                nc.vector.tensor_copy(out=so[:, c0 - d0:c0 - d0 + vsz],
                                      in_=ph[:, :vsz])
        eng = nc.sync if (n + d0 // DSL) % 2 == 0 else nc.gpsimd
        eng.dma_start(out=t_out[n * 128:(n + 1) * 128, d0:d0 + dsz],
                      in_=so[:, :dsz])

    def lm_head_block(n, hfT):
        for d0 in range(0, V, DSL):
            lm_slice(n, hfT, d0)

    def pump(k):
        for _ in range(min(k, len(lm_pending))):
            n, hfT, d0 = lm_pending.pop(0)
            lm_slice(n, hfT, d0)

    def stage_lnqkv(l, s, half):
        blocks = [s * NSB + half * 4 + k for k in range(4)]
        hs = ln_norm_blocks(blocks, f"l{l}a{s}{half}")
        for k, n in enumerate(blocks):
            transpose_gb(hT_all[:, n * 128:(n + 1) * 128], hs[k],
                         g1c[l], b1c[l])
        g = s * 2 + half
        sl5 = slice(g * 512, (g + 1) * 512)
        qc = q_col(l, s, half * 4)
        slq = slice(qc, qc + 512)
        pq = ps512.tile([128, 512], F32, name="pq", tag="ps512")
        nc.tensor.matmul(pq, lhsT=wq_sb[l], rhs=hT_all[:, sl5],
                         start=True, stop=True)
        nc.scalar.copy(out=qT_p[0][:, slq], in_=pq[0:64, :])
        nc.vector.tensor_copy(out=qT_p[1][:, slq], in_=pq[64:128, :])
        pk = ps512.tile([128, 512], F32, name="pk", tag="ps512")
        nc.tensor.matmul(pk, lhsT=wk_sb[l], rhs=hT_all[:, sl5],
                         start=True, stop=True)
        nc.scalar.copy(out=kT_p[0][:, slq], in_=pk[0:64, :])
        nc.vector.tensor_copy(out=kT_p[1][:, slq], in_=pk[64:128, :])
        for k, n in enumerate(blocks):
            pv = ps128.tile([128, 128], F32, name="pv", tag="ps128")
            nc.tensor.matmul(pv, lhsT=hT_all[:, n * 128:(n + 1) * 128],
                             rhs=wv_sb[l], start=True, stop=True)
            nc.scalar.copy(out=v_ext[:, v_slot(l, s, half * 4 + k), :, 0:HS],
                           in_=pv.rearrange("p (h e) -> p h e", e=HS))

    def stage_attention(l, s, i_lo, i_hi, att_all, pump_fn=None):
        """Causal attention for q blocks [i_lo, i_hi] of sequence s."""
        ni = i_hi - i_lo + 1
        e_w = sum((i_hi + 1 - max(j, i_lo)) * 128 for j in range(i_hi + 1))
        for hh in range(H):
            pb = 32 * (hh % 2)
            e_all = ep.tile([128, e_w], F32, name="e_all", tag="e")
            offs = {}
            off = 0
            for j in range(i_hi + 1):
                jq = max(j, i_lo)
                width = (i_hi + 1 - jq) * 128
                offs[j] = (off, jq)
                q0 = q_col(l, s, jq)
                kc = q_col(l, s, j)
                for c0 in range(0, width, 512):
                    csz = min(512, width - c0)
                    sc = pssc.tile([128, 512], F32, name="sc", tag="pssc")
                    nc.tensor.matmul(
                        sc[:, :csz],
                        lhsT=kT_p[hh // 2][pb:pb + 32, kc:kc + 128],
                        rhs=qT_p[hh // 2][pb:pb + 32, q0 + c0:q0 + c0 + csz],
                        start=True, stop=True)
                    nc.scalar.activation(out=e_all[:, off + c0:off + c0 + csz],
                                         in_=sc[:, :csz], func=AF.Exp)
                if jq == j:
                    # zero below-diagonal of the diagonal block, post-exp
                    nc.gpsimd.tensor_mul(out=e_all[:, off:off + 128],
                                         in0=e_all[:, off:off + 128], in1=cm_sb)
                off += width
            aps = psat.tile([128, ni, HS + 1], F32, name="aps", tag="aps")
            n_mm = sum(i + 1 for i in range(i_lo, i_hi + 1))
            k_mm = 0
            for i in range(i_lo, i_hi + 1):
                for j in range(i + 1):
                    o, jq = offs[j]
                    nc.tensor.matmul(
                        aps[:, i - i_lo, :],
                        lhsT=e_all[:, o + (i - jq) * 128:o + (i - jq) * 128 + 128],
                        rhs=v_ext[:, v_slot(l, s, j), hh, :],
                        start=(k_mm == 0), stop=(k_mm == n_mm - 1))
                    k_mm += 1
            zr = wp.tile([128, ni, 1], F32, name="zr", tag="zr")
            nc.vector.reciprocal(out=zr[:, :, 0], in_=aps[:, :, HS])
            for i in range(i_lo, i_hi + 1):
                nc.vector.tensor_scalar_mul(
                    out=att_all[:, i - i_lo, HS * hh:HS * hh + HS],
                    in0=aps[:, i - i_lo, 0:HS], scalar1=zr[:, i - i_lo, :])
            if pump_fn is not None:
                pump_fn()
        for i in range(i_lo, i_hi + 1):
            nq = s * NSB + i
            attT = wp.tile([128, C], F32, name="attT")
            transpose_to(attT, att_all[:, i - i_lo, :], i % 2)
            pr = ps128.tile([128, 128], F32, name="pr", tag="ps128")
            nc.tensor.matmul(pr, lhsT=attT, rhs=wpj_sb[l], start=True,
                             stop=False)
            nc.tensor.matmul(pr, lhsT=ones1, rhs=bp_sb[l], start=False,
                             stop=True)
            nc.vector.tensor_add(out=x_all[:, nq, :], in0=x_all[:, nq, :],
                                 in1=pr)

    def stage_ff(l, s, half, lm_tail):
        g = s * 2 + half
        blocks = [g * 4 + nn for nn in range(4)]
        hs = ln_norm_blocks(blocks, f"l{l}b{s}{half}")
        for k, n in enumerate(blocks):
            transpose_gb(hT_all[:, n * 128:(n + 1) * 128], hs[k],
                         g2c[l], b2c[l])
        sl5 = slice(g * 512, (g + 1) * 512)
        ff1q = fp.tile([128, 4, 512], F32, name="ff1q", tag="ff1")
        for sl in range(4):
            pf = ps512.tile([128, 512], F32, name="pf", tag="ps512")
            nc.tensor.matmul(pf, lhsT=w1_sb[l][:, sl * 128:(sl + 1) * 128],
                             rhs=hT_all[:, sl5], start=True, stop=True)
            # relu(x + bias) as (x add b) max 0 (per-partition bias)
            nc.scalar.activation(out=ff1q[:, sl, :], in_=pf, func=AF.Relu,
                                 bias=bf1_sb[l][:, sl:sl + 1], scale=1.0)
        for nn in range(4):
            n = g * 4 + nn
            p2 = ps128.tile([128, 128], F32, name="p2", tag="ps128")
            for sl in range(4):
                nc.tensor.matmul(p2, lhsT=ff1q[:, sl, nn * 128:(nn + 1) * 128],
                                 rhs=w2_sb[l][:, sl, :],
                                 start=(sl == 0), stop=False)
            nc.tensor.matmul(p2, lhsT=ones1, rhs=bf2_sb[l], start=False,
                             stop=True)
            nc.vector.tensor_add(out=x_all[:, n, :], in0=x_all[:, n, :],
                                 in1=p2)
        if lm_tail == "defer":
            hfs = ln_norm_blocks(blocks, f"f{g}")
            for k, n in enumerate(blocks):
                hfT = hp.tile([128, C], LDT, name="hfT", tag="hfT")
                transpose_gb(hfT, hfs[k], gfc, bfc)
                for d0 in range(0, V, DSL):
                    lm_pending.append((n, hfT, d0))
        elif lm_tail == "inline":
            hfs = ln_norm_blocks(blocks, f"f{g}")
            for k, n in enumerate(blocks):
                hfT = hp.tile([128, C], LDT, name="hfT", tag="hfT")
                transpose_gb(hfT, hfs[k], gfc, bfc)
                lm_head_block(n, hfT)

    # ---- driver: wavefront over (layer, half) per sequence so the first
    # lm_head logits appear ~1/4 into the kernel; deferred lm_head blocks
    # are pumped between stages to keep the store stream fed ----
    for s in range(BL):
        final = s == BL - 1
        for l, half in ((0, 0), (1, 0), (0, 1), (1, 1)):
            lm = (l == L - 1) and ("inline" if (final and half == 1) else "defer")
            stage_lnqkv(l, s, half)
            pump(3)
            att = ap_.tile([128, 4, C], F32, name="att_all", tag="att")
            stage_attention(l, s, half * 4, half * 4 + 3, att,
                            pump_fn=lambda: pump(2))
            stage_ff(l, s, half, lm_tail=lm)
            pump(3)
    pump(len(lm_pending))
    stack.close()


def build_module(meas_iters=0):
    global _CACHED_NC
    if _CACHED_NC is not None and meas_iters == 0:
        return _CACHED_NC
    PDT = F32R if SCORES_F32R else F32
    LDT = F32R if LM_MODE == "f32r" else F32
    nc = bacc.Bacc("TRN2", target_bir_lowering=False, debug=False)
    t_in = {}
    t_in["idx"] = nc.declare_dram_parameter("idx", [128, NB], I32, isOutput=False)
    t_in["tok_emb"] = nc.declare_dram_parameter("tok_emb", [V, C], F32, isOutput=False)
    t_in["pos"] = nc.declare_dram_parameter("pos", [128, NB, C], F32, isOutput=False)
    t_in["cmask"] = nc.declare_dram_parameter("cmask", [128, 128], F32, isOutput=False)
    for l in range(L):
        for nm, shp, dt in ((f"wq{l}", [C, C], PDT), (f"wk{l}", [C, C], PDT),
                            (f"wv{l}", [C, C], PDT), (f"wp{l}", [C, C], F32),
                            (f"w1{l}", [C, FF], PDT), (f"w2{l}", [C, 4, C], F32),
                            (f"bf1{l}", [C, 4], F32), (f"bp{l}", [1, C], F32),
                            (f"bf2{l}", [1, C], F32), (f"g1{l}", [C], F32),
                            (f"b1{l}", [C], F32), (f"g2{l}", [C], F32),
                            (f"b2{l}", [C], F32)):
            t_in[nm] = nc.declare_dram_parameter(nm, shp, dt, isOutput=False)
    t_in["gf"] = nc.declare_dram_parameter("gf", [C], F32, isOutput=False)
    t_in["bf"] = nc.declare_dram_parameter("bf", [C], F32, isOutput=False)
    t_in["wh"] = nc.declare_dram_parameter("wh", [C, V], LDT, isOutput=False)
    t_out = nc.declare_dram_parameter("out", [NTOK, V], F32, isOutput=True)

    with TileContext(nc) as tc:
        if meas_iters > 0:
            with tc.For_i(0, meas_iters, 1):
                _emit(nc, tc, t_in, t_out)
        else:
            _emit(nc, tc, t_in, t_out)
    nc.finalize()
    if meas_iters == 0:
        _CACHED_NC = nc
    return nc


def prep_inputs(inputs):
    """Host-side shard prep: returns (in_maps for 8 cores, b_head)."""
    f32 = lambda a: np.ascontiguousarray(np.asarray(a), dtype=np.float32)
    idx = np.asarray(inputs["idx"]).astype(np.int32)

    shared = {
        "tok_emb": f32(inputs["tok_emb"]),
        "pos": np.tile(f32(inputs["pos_emb"])[:T].reshape(NSB, 128, C)
                       .transpose(1, 0, 2), (1, BL, 1)).copy(),
        "cmask": (np.arange(128)[None, :] >= np.arange(128)[:, None]
                  ).astype(np.float32),
        "gf": f32(inputs["lnf_g"]),
        "bf": f32(inputs["lnf_b"]),
        "wh": f32(inputs["w_head"]),
    }
    wq, wk, wv = inputs["wq"], inputs["wk"], inputs["wv"]
    for l in range(L):
        # attention scale folded into wq so q comes out pre-scaled
        shared[f"wq{l}"] = f32(np.transpose(np.asarray(wq)[l], (1, 0, 2))
                               .reshape(C, C) * np.float32(SCALE))
        shared[f"wk{l}"] = f32(np.transpose(np.asarray(wk)[l], (1, 0, 2)).reshape(C, C))
        shared[f"wv{l}"] = f32(np.transpose(np.asarray(wv)[l], (1, 0, 2)).reshape(C, C))
        shared[f"wp{l}"] = f32(inputs["w_proj"][l])
        shared[f"w1{l}"] = f32(inputs["w_ff1"][l])
        shared[f"w2{l}"] = f32(np.asarray(inputs["w_ff2"][l]).reshape(4, 128, C)
                               .transpose(1, 0, 2))
        shared[f"bf1{l}"] = f32(np.asarray(inputs["b_ff1"][l]).reshape(4, 128).T)
        shared[f"bp{l}"] = f32(inputs["b_proj"][l]).reshape(1, C)
        shared[f"bf2{l}"] = f32(inputs["b_ff2"][l]).reshape(1, C)
        shared[f"g1{l}"] = f32(inputs["ln1_g"][l])
        shared[f"b1{l}"] = f32(inputs["ln1_b"][l])
        shared[f"g2{l}"] = f32(inputs["ln2_g"][l])
        shared[f"b2{l}"] = f32(inputs["ln2_b"][l])

    in_maps = []
    for c in range(NCORES):
        shard = idx[c * BL:(c + 1) * BL].reshape(NTOK)
        m = dict(shared)
        m["idx"] = np.ascontiguousarray(shard.reshape(NB, 128).T)
        in_maps.append(m)
    return in_maps, np.asarray(inputs["b_head"], dtype=np.float32)


def kernel(**inputs):
    nc = build_module()
    in_maps, b_head = prep_inputs(inputs)
    res = run_bass_kernel_spmd(nc, in_maps, core_ids=list(range(NCORES)))
    out = np.concatenate([r["out"] for r in res.results], axis=0)
    out = out.reshape(B, T, V)
    if np.any(b_head):
        out = out + b_head
    return out
